# revision 44
# baseline (speedup 1.0000x reference)
"""Trainium2 Bass kernel for nn_ChannelLatencySeq2Value.

Data-parallel over B across 8 NeuronCores (2 batch rows / core).

Per core:
  - dendritic_drive (4 depthwise conv paths) runs on the TensorEngine as
    block-diagonal bf16 matmuls in [c, (b, t)] layout. Tap pairs are packed
    onto the 128-row contraction dim by storing a one-column-shifted copy of
    x on partitions 64..127, so the 16 taps collapse into 5 matmul passes.
    ScalarE copies PSUM->SBUF adding the conv bias; results DMA straight out.
  - summed (the LIF drive) stays exact fp32 on VectorE in [(b,c), t] layout:
    a 5-tap effective kernel (conv paths pre-mixed with reduce_w host-side),
    then one tensor_tensor_scan for the LIF recurrence, a running-max scan,
    and a fused compare+accumulate for the first-crossing latency.
  - act/gates/MLP run as [(b, feature), 1] column matmuls (block-diagonal
    lhsT packs), softplus via exp + a degree-6 ln(1+u) polynomial on VectorE
    so only one ACT table set is ever loaded.

All parameters are tiny and replicated; no collectives are needed.
Measured: ~88-91 us HW exec (8 cores), outputs <= 2e-3 rel err (bf16 conv),
true_latency bit-exact vs an fp32 reference scan.
"""

import numpy as np

import concourse.bass as bass
import concourse.bacc as bacc
import concourse.mybir as mybir
from concourse import tile
from concourse.bass_utils import run_bass_kernel_spmd

F32 = mybir.dt.float32
BF16 = mybir.dt.bfloat16
OP = mybir.AluOpType
AF = mybir.ActivationFunctionType

B, C, T = 16, 64, 4096
K1, O1 = 3, 2
K2, O2 = 5, 2
D = O1 + O2
DEC = 128
TAU, THRESH = 5.0, 1.0
ALPHA = float(np.exp(-1.0 / TAU))
N_CORES = 8
NB = B // N_CORES          # batch rows per core
N = NB * C                 # partition rows per core (= 128)
TP = T + 4                 # per-b padded width (2 zeros each side)
FCH = 512                  # matmul moving chunk (one PSUM bank)
HCH = 1024                 # staging chunk (2 banks; psA+psB double-buffered)

# pp column layout (per-partition parameter pack)
PP_EW = 0                  # effective summed kernel taps, cols 0..4
PP_EB = 5                  # effective summed bias
PP_MB1 = 6                 # mlp_b1 (row = DEC index)
PP_MB2 = 7                 # mlp_b2 (rows 0..C-1)
PP_BA = 8                  # conv bias for A-set rows (dd0|dd1)
PP_BB = 9                  # conv bias for B-set rows (dd2|dd3)
PP_NCOL = 10

# matmul tap shift bases (rhs col offset = 2 + t0 + s)
MM_SHIFT = [-1, 1, -2, 0, 2]   # mm0,mm1 -> A-set; mm2..mm4 -> B-set
MM_SET = [0, 0, 1, 1, 1]

_GRAPH_CACHE = {}


def _build_graph(scale_val: float):
    nc = bacc.Bacc("TRN2", target_bir_lowering=False, debug=False,
                   num_devices=N_CORES)

    x_in = nc.declare_dram_parameter("x", [N, T], F32, isOutput=False)
    pp_in = nc.declare_dram_parameter("pp", [N, PP_NCOL], F32, isOutput=False)
    wm_in = nc.declare_dram_parameter("wm", [N, 448], F32, isOutput=False)
    wc_in = nc.declare_dram_parameter("wc", [N, 10 * 128], F32, isOutput=False)
    dd_out = nc.declare_dram_parameter("dd", [N, D, T], F32, isOutput=True)
    lat_out = nc.declare_dram_parameter("lat", [N, 1], F32, isOutput=True)
    act_out = nc.declare_dram_parameter("act", [N, 1], F32, isOutput=True)
    pred_out = nc.declare_dram_parameter("pred", [N, 1], F32, isOutput=True)

    HLF = T // 2
    with tile.TileContext(nc) as tc:
        with (
            tc.tile_pool(name="pool", bufs=1) as pool,
            tc.tile_pool(name="stage", bufs=4) as stage,
            tc.tile_pool(name="psum", bufs=2, space="PSUM") as psum,
        ):
            # ---------- input DMAs (one queue, priority order) ----------
            pp = pool.tile([N, PP_NCOL], F32, tag="pp")
            wm = pool.tile([N, 448], F32, tag="wm")
            wcf = pool.tile([N, 10 * 128], F32, tag="wcf")
            nc.sync.dma_start(out=pp[:, :], in_=pp_in[:, :])
            nc.sync.dma_start(out=wcf[:, :], in_=wc_in[:, :])

            xsA = pool.tile([N, HLF + 4], F32, tag="xsA")
            xsB = pool.tile([N, HLF + 4], F32, tag="xsB")
            nc.vector.memset(xsA[:, 0:2], 0.0)
            nc.sync.dma_start(out=xsA[:, 2:HLF + 4], in_=x_in[:, 0:HLF + 2])
            nc.scalar.dma_start(out=xsB[:, 0:HLF + 2], in_=x_in[:, HLF - 2:T])
            nc.vector.memset(xsB[:, HLF + 2:HLF + 4], 0.0)
            nc.sync.dma_start(out=wm[:, :], in_=wm_in[:, :])

            # warm the exp ACT table set (covers identity/abs/relu/copy too);
            # no other set is ever needed, so no tail table switches
            warm = pool.tile([N, 1], F32, tag="warm")
            nc.vector.memset(warm[:, :], 0.0)
            nc.scalar.activation(out=warm[:, :], in_=warm[:, :], func=AF.Exp)
            wcb = pool.tile([N, 10 * 128], BF16, tag="wcb")
            nc.scalar.copy(out=wcb[:, :], in_=wcf[:, :])

            # ---------- x2b built straight from the xs halves -------------
            # x2b0: rows 0..63 = b0 channels (cast), rows 64..127 = the same
            # shifted left one column (SB2SB). x2b1 mirrored: plain rows live
            # on 64..127 (partition-aligned with xs), shifted copy on 0..63;
            # its matmuls use the row-swapped weight stack.
            s = pool.tile([N, T], F32, tag="s")
            x2bs = []
            XH = HLF + 4  # xsA/xsB width; cast1 covers cols [0, XH)
            for b in range(NB):
                x2b = pool.tile([N, TP], BF16, tag=f"x2b{b}", name=f"x2b{b}")
                src = slice(0, C) if b == 0 else slice(C, N)
                pl = slice(0, C) if b == 0 else slice(C, N)   # plain rows
                sh = slice(C, N) if b == 0 else slice(0, C)   # shifted rows
                nc.scalar.copy(out=x2b[pl, 0:XH], in_=xsA[src, 0:XH])
                nc.scalar.copy(out=x2b[pl, XH:TP], in_=xsB[src, 4:HLF + 4])
                nc.vector.memset(x2b[sh, TP - 1:TP], 0.0)
                nc.scalar.dma_start(out=x2b[sh, 0:XH - 1],
                                      in_=x2b[pl, 1:XH])
                nc.scalar.dma_start(out=x2b[sh, XH - 1:TP - 1],
                                      in_=x2b[pl, XH:TP])
                x2bs.append(x2b)
                xsh, o = ((xsA, 0), (xsB, HLF))[b]
                nc.scalar.activation(
                    out=s[:, o:o + HLF], in_=xsh[:, 0:HLF], func=AF.Identity,
                    bias=pp[:, PP_EB:PP_EB + 1], scale=pp[:, PP_EW:PP_EW + 1])

            alph = pool.tile([N, T], F32, tag="alph")
            nc.gpsimd.memset(alph[:, :], ALPHA)

            # ---------- summed accumulating taps (fp32, VectorE) ----------
            for xsh, o in ((xsA, 0), (xsB, HLF)):
                for j in range(1, 5):
                    nc.vector.scalar_tensor_tensor(
                        out=s[:, o:o + HLF], in0=xsh[:, j:j + HLF],
                        scalar=pp[:, PP_EW + j:PP_EW + j + 1],
                        in1=s[:, o:o + HLF], op0=OP.mult, op1=OP.add)

            # ---------- dendritic drive via TensorE ----------
            # groups of (b, T-half); stores stream out per group so the
            # 8MiB writeback overlaps the rest of the matmul work
            THL = T // 2
            dd_n = 0
            for b, th in ((0, 0), (1, 0), (0, 1), (1, 1)):
                sbs = [stage.tile([N, THL], F32, tag=f"stg{si}",
                                  name=f"ddsb{b}{th}{si}") for si in range(2)]
                for hh in range(THL // HCH):
                    t0 = th * THL + hh * HCH
                    psA = psum.tile([N, HCH], F32, tag="psA", name="psA")
                    psB = psum.tile([N, HCH], F32, tag="psB", name="psB")
                    for q in range(HCH // FCH):
                        for mm in range(5):
                            ps = psA if MM_SET[mm] == 0 else psB
                            first = mm == 0 or mm == 2
                            last = mm == 1 or mm == 4
                            s_ = MM_SHIFT[mm]
                            c0 = 2 + t0 + q * FCH + s_
                            mmi = mm + 5 * b
                            if mm in (1, 4):
                                kr = slice(0, C) if b == 0 else slice(C, N)
                            else:
                                kr = slice(0, N)
                            nc.tensor.matmul(
                                ps[:, q * FCH:(q + 1) * FCH],
                                wcb[kr, mmi * 128:(mmi + 1) * 128],
                                x2bs[b][kr, c0:c0 + FCH],
                                start=first, stop=last)
                    for si, ps, bcol in ((0, psA, PP_BA), (1, psB, PP_BB)):
                        nc.scalar.activation(
                            out=sbs[si][:, hh * HCH:(hh + 1) * HCH],
                            in_=ps[:, :], func=AF.Identity,
                            bias=pp[:, bcol:bcol + 1], scale=1.0)
                for si in range(2):
                    for phalf in range(2):
                        eng = nc.sync if dd_n % 2 == 0 else nc.scalar
                        dd_n += 1
                        eng.dma_start(
                            out=dd_out[b * C:(b + 1) * C, 2 * si + phalf,
                                       th * THL:(th + 1) * THL],
                            in_=sbs[si][phalf * C:(phalf + 1) * C, :])

            # ---------- LIF scan + crossing ----------
            V = pool.tile([N, T], F32, tag="V")
            nc.vector.tensor_tensor_scan(
                out=V[:, :], data0=alph[:, :], data1=s[:, :], initial=0.0,
                op0=OP.mult, op1=OP.add)
            rm = pool.tile([N, T], F32, tag="rm")
            nc.vector.tensor_tensor_scan(
                out=rm[:, :], data0=V[:, :], data1=V[:, :], initial=-3.0e38,
                op0=OP.max, op1=OP.max)
            lat = pool.tile([N, 1], F32, tag="lat")
            nc.vector.tensor_scalar(
                out=rm[:, :], in0=rm[:, :], scalar1=THRESH, scalar2=0.0,
                op0=OP.is_lt, op1=OP.add, accum_out=lat[:, :])
            nc.scalar.dma_start(out=lat_out[:, :], in_=lat[:, :])

            # ---------- act = exp(-lat/scale), gates + MLP ----------
            # whole MLP stays in [(b, feature), 1] column layout: contraction
            # runs over partitions via block-diagonal / half-zero lhsT packs.
            actv = pool.tile([N, 1], F32, tag="actv")
            nc.scalar.activation(out=actv[:, :], in_=lat[:, :], func=AF.Exp,
                                 scale=-1.0 / scale_val)
            nc.scalar.dma_start(out=act_out[:, :], in_=actv[:, :])

            ps_m = psum.tile([N, 1], F32, tag="psA", name="ps_m")
            nc.tensor.matmul(ps_m[:, :], wm[:, 0:128], actv[:, :],
                             start=True, stop=True)
            mixv = pool.tile([N, 1], F32, tag="mixv")
            nc.scalar.copy(out=mixv[:, :], in_=ps_m[:, :])
            hs = []
            for b in range(NB):
                ps_h = psum.tile([DEC, 1], F32, tag="psB", name=f"ps_h{b}")
                nc.tensor.matmul(ps_h[:, :], wm[:, 128 + b * 128:256 + b * 128],
                                 mixv[:, :], start=True, stop=True)
                hv = pool.tile([DEC, 1], F32, tag=f"hv{b}", name=f"hv{b}")
                nc.scalar.activation(out=hv[:, :], in_=ps_h[:, :],
                                     func=AF.Relu,
                                     bias=pp[:, PP_MB1:PP_MB1 + 1], scale=1.0)
                hs.append(hv)
            ps_r = psum.tile([N, 1], F32, tag="psA", name="ps_r")
            for b in range(NB):
                nc.tensor.matmul(ps_r[b * C:(b + 1) * C, :], wm[:, 384:448],
                                 hs[b][:, :], start=True, stop=True)
            # softplus(raw) = relu(raw) + P(exp(-|raw|)), raw = psum + b2,
            # with P a degree-6 minimax fit of ln(1+u) on [0,1] (err ~1.5e-6)
            # evaluated on VectorE so no Ln table is ever loaded
            PC = [1.4698117483186821e-06, 0.9998477529839347,
                  -0.4973735992304131, 0.31574842159222594,
                  -0.1903558305283913, 0.0826921540715837,
                  -0.01741427410397167]
            absT = pool.tile([N, 1], F32, tag="absT")
            nc.scalar.activation(out=absT[:, :], in_=ps_r[:, :], func=AF.Abs,
                                 bias=pp[:, PP_MB2:PP_MB2 + 1], scale=1.0)
            expT = pool.tile([N, 1], F32, tag="expT")
            nc.scalar.activation(out=expT[:, :], in_=absT[:, :], func=AF.Exp,
                                 scale=-1.0)
            reluT = pool.tile([N, 1], F32, tag="reluT")
            nc.vector.tensor_scalar(
                out=reluT[:, :], in0=ps_r[:, :],
                scalar1=pp[:, PP_MB2:PP_MB2 + 1], scalar2=0.0,
                op0=OP.add, op1=OP.max)
            ply = pool.tile([N, 1], F32, tag="ply")
            nc.vector.tensor_scalar(out=ply[:, :], in0=expT[:, :],
                                    scalar1=PC[6], scalar2=PC[5],
                                    op0=OP.mult, op1=OP.add)
            for k in (4, 3, 2, 1, 0):
                nc.vector.tensor_mul(out=ply[:, :], in0=ply[:, :],
                                     in1=expT[:, :])
                nc.vector.tensor_scalar_add(out=ply[:, :], in0=ply[:, :],
                                            scalar1=PC[k])
            predT = pool.tile([N, 1], F32, tag="predT")
            nc.vector.tensor_add(out=predT[:, :], in0=reluT[:, :],
                                 in1=ply[:, :])
            nc.vector.tensor_scalar(
                out=predT[:, :], in0=predT[:, :], scalar1=0.0, scalar2=float(T),
                op0=OP.max, op1=OP.min)
            nc.scalar.dma_start(out=pred_out[:, :], in_=predT[:, :])

    nc.compile()
    return nc


def _host_prep(w3, b3, w5, b5, reduce_w, reduce_b, output_gates,
               mlp_w1, mlp_b1, mlp_w2, mlp_b2):
    cidx = np.arange(C)
    one_m_a = np.float32(1.0 - ALPHA)

    def per_row(vec_c):
        return np.tile(np.asarray(vec_c, np.float32), NB)

    # effective 5-tap summed kernel: conv paths mixed with reduce_w, then
    # scaled by (1-alpha) so the scan's data1 is the scaled drive directly
    eff_w = np.zeros((C, 5), np.float32)
    # conv3 taps sit at positions 1..3 of the 5-tap window
    for j in range(K1):
        eff_w[:, j + 1] += reduce_w[:, 0] * w3[2 * cidx + 0, 0, j]
        eff_w[:, j + 1] += reduce_w[:, 1] * w3[2 * cidx + 1, 0, j]
    for j in range(K2):
        eff_w[:, j] += reduce_w[:, 2] * w5[2 * cidx + 0, 0, j]
        eff_w[:, j] += reduce_w[:, 3] * w5[2 * cidx + 1, 0, j]
    eff_b = (reduce_w[:, 0] * b3[2 * cidx + 0] +
             reduce_w[:, 1] * b3[2 * cidx + 1] +
             reduce_w[:, 2] * b5[2 * cidx + 0] +
             reduce_w[:, 3] * b5[2 * cidx + 1] + reduce_b)
    eff_w *= one_m_a
    eff_b = eff_b * one_m_a

    pp = np.zeros((N, PP_NCOL), np.float32)
    for j in range(5):
        pp[:, PP_EW + j] = per_row(eff_w[:, j])
    pp[:, PP_EB] = per_row(eff_b)
    pp[:, PP_MB1] = np.asarray(mlp_b1, np.float32)
    pp[:, PP_MB2] = per_row(mlp_b2)
    pp[0:C, PP_BA] = b3[2 * cidx + 0]
    pp[C:N, PP_BA] = b3[2 * cidx + 1]
    pp[0:C, PP_BB] = b5[2 * cidx + 0]
    pp[C:N, PP_BB] = b5[2 * cidx + 1]

    # MLP lhsT packs for the [(b, feature), 1] column formulation:
    #   cols 0:128    block-diag over b of output_gates.T  (act -> mixed)
    #   cols 128:256  w1.T on rows (b=0, ci), zeros elsewhere
    #   cols 256:384  w1.T on rows (b=1, ci)
    #   cols 384:448  w2.T (contraction over all DEC rows)
    wm = np.zeros((N, 448), np.float32)
    gT = np.ascontiguousarray(output_gates.T)
    wm[0:C, 0:C] = gT
    wm[C:N, C:2 * C] = gT
    wm[0:C, 128:256] = np.ascontiguousarray(mlp_w1.T)
    wm[C:N, 256:384] = np.ascontiguousarray(mlp_w1.T)
    wm[:, 384:448] = np.ascontiguousarray(mlp_w2.T)

    # conv matmul lhsT stack [128, 10*128]: out = lhsT.T @ rhs.
    # Blocks 0..4 (b=0): rhs row k<64 is x[b, k, t+s], row 64+k the
    # one-shifted copy. Blocks 5..9 (b=1) are the row-half-swapped
    # versions (plain rows live on partitions 64..127 for b=1).
    # out col m<64: path A0/B0 channel m; col 64+m: path A1/B1 channel m.
    wc = np.zeros((N, 10 * 128), np.float32)

    def put(mm, krow, mcol, val):
        wc[krow, mm * 128 + mcol] = val

    for c in range(C):
        # mm0: A-set, s=-1: top tap j=0, bottom tap j=1
        put(0, c, c, w3[2 * c + 0, 0, 0])
        put(0, c, 64 + c, w3[2 * c + 1, 0, 0])
        put(0, 64 + c, c, w3[2 * c + 0, 0, 1])
        put(0, 64 + c, 64 + c, w3[2 * c + 1, 0, 1])
        # mm1: A-set, s=+1: top tap j=2
        put(1, c, c, w3[2 * c + 0, 0, 2])
        put(1, c, 64 + c, w3[2 * c + 1, 0, 2])
        # mm2: B-set, s=-2: top j=0, bottom j=1
        put(2, c, c, w5[2 * c + 0, 0, 0])
        put(2, c, 64 + c, w5[2 * c + 1, 0, 0])
        put(2, 64 + c, c, w5[2 * c + 0, 0, 1])
        put(2, 64 + c, 64 + c, w5[2 * c + 1, 0, 1])
        # mm3: B-set, s=0: top j=2, bottom j=3
        put(3, c, c, w5[2 * c + 0, 0, 2])
        put(3, c, 64 + c, w5[2 * c + 1, 0, 2])
        put(3, 64 + c, c, w5[2 * c + 0, 0, 3])
        put(3, 64 + c, 64 + c, w5[2 * c + 1, 0, 3])
        # mm4: B-set, s=+2: top j=4
        put(4, c, c, w5[2 * c + 0, 0, 4])
        put(4, c, 64 + c, w5[2 * c + 1, 0, 4])
    # b=1 blocks: swap the row halves of each lhsT
    for mm in range(5):
        blk = wc[:, mm * 128:(mm + 1) * 128]
        wc[0:C, (5 + mm) * 128:(6 + mm) * 128] = blk[C:N, :]
        wc[C:N, (5 + mm) * 128:(6 + mm) * 128] = blk[0:C, :]

    return pp, wm, wc


def _run(inputs, trace=False):
    x = np.asarray(inputs["x"], np.float32)
    scale_val = max(float(np.asarray(inputs["latency_scale"])), 0.001)
    pp, wm, wc = _host_prep(
        np.asarray(inputs["w3"], np.float32), np.asarray(inputs["b3"], np.float32),
        np.asarray(inputs["w5"], np.float32), np.asarray(inputs["b5"], np.float32),
        np.asarray(inputs["reduce_w"], np.float32),
        np.asarray(inputs["reduce_b"], np.float32),
        np.asarray(inputs["output_gates"], np.float32),
        np.asarray(inputs["mlp_w1"], np.float32),
        np.asarray(inputs["mlp_b1"], np.float32),
        np.asarray(inputs["mlp_w2"], np.float32),
        np.asarray(inputs["mlp_b2"], np.float32))

    key = round(scale_val, 9)
    if key not in _GRAPH_CACHE:
        _GRAPH_CACHE[key] = _build_graph(scale_val)
    nc = _GRAPH_CACHE[key]

    in_maps = []
    for k in range(N_CORES):
        xs = np.ascontiguousarray(
            x[k * NB:(k + 1) * NB].reshape(N, T), dtype=np.float32)
        in_maps.append({"x": xs, "pp": pp, "wm": wm, "wc": wc})

    res = run_bass_kernel_spmd(nc, in_maps, list(range(N_CORES)), trace=trace)

    pred = np.concatenate(
        [res.results[k]["pred"].reshape(NB, C) for k in range(N_CORES)], axis=0)
    lat = np.concatenate(
        [res.results[k]["lat"].reshape(NB, C) for k in range(N_CORES)], axis=0)
    act = np.concatenate(
        [res.results[k]["act"].reshape(NB, C) for k in range(N_CORES)], axis=0)
    dd = np.concatenate(
        [res.results[k]["dd"].reshape(NB, C, D, T) for k in range(N_CORES)],
        axis=0)
    outs = (pred.astype(np.float32), lat.astype(np.float32),
            act.astype(np.float32), dd.astype(np.float32))
    return outs, res


def kernel(**inputs):
    outs, _ = _run(inputs, trace=False)
    return outs


# revision 45
# speedup vs baseline: 1.1354x; 1.1354x over previous
"""Trainium2 Bass kernel for nn_ChannelLatencySeq2Value.

Data-parallel over B across 8 NeuronCores (2 batch rows / core).

Per core:
  - dendritic_drive (4 depthwise conv paths) runs on the TensorEngine as
    block-diagonal bf16 matmuls in [c, (b, t)] layout. Tap pairs are packed
    onto the 128-row contraction dim by storing a one-column-shifted copy of
    x on partitions 64..127, so the 16 taps collapse into 5 matmul passes.
    ScalarE copies PSUM->SBUF adding the conv bias; results DMA straight out.
  - summed (the LIF drive) stays exact fp32 on VectorE in [(b,c), t] layout:
    a 5-tap effective kernel (conv paths pre-mixed with reduce_w host-side),
    then one tensor_tensor_scan for the LIF recurrence, a running-max scan,
    and a fused compare+accumulate for the first-crossing latency.
  - act/gates/MLP run as [(b, feature), 1] column matmuls (block-diagonal
    lhsT packs), softplus via exp + a degree-6 ln(1+u) polynomial on VectorE
    so only one ACT table set is ever loaded.

All parameters are tiny and replicated; no collectives are needed.
Measured: ~88-91 us HW exec (8 cores), outputs <= 2e-3 rel err (bf16 conv),
true_latency bit-exact vs an fp32 reference scan.
"""

import numpy as np

import concourse.bass as bass
import concourse.bacc as bacc
import concourse.mybir as mybir
from concourse import tile
from concourse.bass_utils import run_bass_kernel_spmd

F32 = mybir.dt.float32
BF16 = mybir.dt.bfloat16
OP = mybir.AluOpType
AF = mybir.ActivationFunctionType

B, C, T = 16, 64, 4096
K1, O1 = 3, 2
K2, O2 = 5, 2
D = O1 + O2
DEC = 128
TAU, THRESH = 5.0, 1.0
ALPHA = float(np.exp(-1.0 / TAU))
N_CORES = 8
NB = B // N_CORES          # batch rows per core
N = NB * C                 # partition rows per core (= 128)
TP = T + 4                 # per-b padded width (2 zeros each side)
FCH = 512                  # matmul moving chunk (one PSUM bank)
HCH = 1024                 # staging chunk (2 banks; psA+psB double-buffered)

# pp column layout (per-partition parameter pack)
PP_EW = 0                  # effective summed kernel taps, cols 0..4
PP_EB = 5                  # effective summed bias
PP_MB1 = 6                 # mlp_b1 (row = DEC index)
PP_MB2 = 7                 # mlp_b2 (rows 0..C-1)
PP_BA = 8                  # conv bias for A-set rows (dd0|dd1)
PP_BB = 9                  # conv bias for B-set rows (dd2|dd3)
PP_NCOL = 10

# matmul tap shift bases (rhs col offset = 2 + t0 + s)
MM_SHIFT = [-1, 1, -2, 0, 2]   # mm0,mm1 -> A-set; mm2..mm4 -> B-set
MM_SET = [0, 0, 1, 1, 1]

_GRAPH_CACHE = {}


def _build_graph(scale_val: float):
    nc = bacc.Bacc("TRN2", target_bir_lowering=False, debug=False,
                   num_devices=N_CORES)

    x_in = nc.declare_dram_parameter("x", [N, T], F32, isOutput=False)
    pp_in = nc.declare_dram_parameter("pp", [N, PP_NCOL], F32, isOutput=False)
    wm_in = nc.declare_dram_parameter("wm", [N, 448], F32, isOutput=False)
    wc_in = nc.declare_dram_parameter("wc", [N, 10 * 128], F32, isOutput=False)
    dd_out = nc.declare_dram_parameter("dd", [N, D, T], F32, isOutput=True)
    lat_out = nc.declare_dram_parameter("lat", [N, 1], F32, isOutput=True)
    act_out = nc.declare_dram_parameter("act", [N, 1], F32, isOutput=True)
    pred_out = nc.declare_dram_parameter("pred", [N, 1], F32, isOutput=True)

    HLF = T // 2
    with tile.TileContext(nc) as tc:
        with (
            tc.tile_pool(name="pool", bufs=1) as pool,
            tc.tile_pool(name="stage", bufs=4) as stage,
            tc.tile_pool(name="psum", bufs=2, space="PSUM") as psum,
        ):
            # ---------- input DMAs (one queue, priority order) ----------
            pp = pool.tile([N, PP_NCOL], F32, tag="pp")
            wm = pool.tile([N, 448], F32, tag="wm")
            wcf = pool.tile([N, 10 * 128], F32, tag="wcf")
            nc.sync.dma_start(out=pp[:, :], in_=pp_in[:, :])
            nc.sync.dma_start(out=wcf[:, :], in_=wc_in[:, :])

            xsA = pool.tile([N, HLF + 4], F32, tag="xsA")
            xsB = pool.tile([N, HLF + 4], F32, tag="xsB")
            nc.vector.memset(xsA[:, 0:2], 0.0)
            nc.sync.dma_start(out=xsA[:, 2:HLF + 4], in_=x_in[:, 0:HLF + 2])
            nc.scalar.dma_start(out=xsB[:, 0:HLF + 2], in_=x_in[:, HLF - 2:T])
            nc.vector.memset(xsB[:, HLF + 2:HLF + 4], 0.0)
            nc.sync.dma_start(out=wm[:, :], in_=wm_in[:, :])

            # warm the exp ACT table set (covers identity/abs/relu/copy too);
            # no other set is ever needed, so no tail table switches
            warm = pool.tile([N, 1], F32, tag="warm")
            nc.vector.memset(warm[:, :], 0.0)
            nc.scalar.activation(out=warm[:, :], in_=warm[:, :], func=AF.Exp)
            wcb = pool.tile([N, 10 * 128], BF16, tag="wcb")
            nc.scalar.copy(out=wcb[:, :], in_=wcf[:, :])

            # ---------- x2b built straight from the xs halves -------------
            # x2b0: rows 0..63 = b0 channels (cast), rows 64..127 = the same
            # shifted left one column (SB2SB). x2b1 mirrored: plain rows live
            # on 64..127 (partition-aligned with xs), shifted copy on 0..63;
            # its matmuls use the row-swapped weight stack.
            s = pool.tile([N, T], F32, tag="s")
            x2bs = []
            XH = HLF + 4  # xsA/xsB width; cast1 covers cols [0, XH)
            for b in range(NB):
                x2b = pool.tile([N, TP], BF16, tag=f"x2b{b}", name=f"x2b{b}")
                src = slice(0, C) if b == 0 else slice(C, N)
                pl = slice(0, C) if b == 0 else slice(C, N)   # plain rows
                sh = slice(C, N) if b == 0 else slice(0, C)   # shifted rows
                nc.scalar.copy(out=x2b[pl, 0:XH], in_=xsA[src, 0:XH])
                nc.scalar.copy(out=x2b[pl, XH:TP], in_=xsB[src, 4:HLF + 4])
                nc.vector.memset(x2b[sh, TP - 1:TP], 0.0)
                nc.scalar.dma_start(out=x2b[sh, 0:XH - 1],
                                      in_=x2b[pl, 1:XH])
                nc.scalar.dma_start(out=x2b[sh, XH - 1:TP - 1],
                                      in_=x2b[pl, XH:TP])
                x2bs.append(x2b)
                xsh, o = ((xsA, 0), (xsB, HLF))[b]
                nc.scalar.activation(
                    out=s[:, o:o + HLF], in_=xsh[:, 0:HLF], func=AF.Identity,
                    bias=pp[:, PP_EB:PP_EB + 1], scale=pp[:, PP_EW:PP_EW + 1])

            alph = pool.tile([N, T], F32, tag="alph")
            nc.gpsimd.memset(alph[:, :], ALPHA)

            # ---------- summed accumulating taps (fp32, VectorE) ----------
            for xsh, o in ((xsA, 0), (xsB, HLF)):
                for j in range(1, 5):
                    nc.vector.scalar_tensor_tensor(
                        out=s[:, o:o + HLF], in0=xsh[:, j:j + HLF],
                        scalar=pp[:, PP_EW + j:PP_EW + j + 1],
                        in1=s[:, o:o + HLF], op0=OP.mult, op1=OP.add)

            # ---------- dendritic drive via TensorE ----------
            # groups of (b, T-half); stores stream out per group so the
            # 8MiB writeback overlaps the rest of the matmul work
            THL = T // 2
            dd_n = 0
            for b, th in ((0, 0), (1, 0), (0, 1), (1, 1)):
                sbs = [stage.tile([N, THL], F32, tag=f"stg{si}",
                                  name=f"ddsb{b}{th}{si}") for si in range(2)]
                for hh in range(THL // HCH):
                    t0 = th * THL + hh * HCH
                    psA = psum.tile([N, HCH], F32, tag="psA", name="psA")
                    psB = psum.tile([N, HCH], F32, tag="psB", name="psB")
                    for q in range(HCH // FCH):
                        for mm in range(5):
                            ps = psA if MM_SET[mm] == 0 else psB
                            first = mm == 0 or mm == 2
                            last = mm == 1 or mm == 4
                            s_ = MM_SHIFT[mm]
                            c0 = 2 + t0 + q * FCH + s_
                            mmi = mm + 5 * b
                            nc.tensor.matmul(
                                ps[:, q * FCH:(q + 1) * FCH],
                                wcb[:, mmi * 128:(mmi + 1) * 128],
                                x2bs[b][:, c0:c0 + FCH],
                                start=first, stop=last)
                    for si, ps, bcol in ((0, psA, PP_BA), (1, psB, PP_BB)):
                        nc.scalar.activation(
                            out=sbs[si][:, hh * HCH:(hh + 1) * HCH],
                            in_=ps[:, :], func=AF.Identity,
                            bias=pp[:, bcol:bcol + 1], scale=1.0)
                for si in range(2):
                    for phalf in range(2):
                        eng = nc.sync if dd_n % 2 == 0 else nc.scalar
                        dd_n += 1
                        eng.dma_start(
                            out=dd_out[b * C:(b + 1) * C, 2 * si + phalf,
                                       th * THL:(th + 1) * THL],
                            in_=sbs[si][phalf * C:(phalf + 1) * C, :])

            # ---------- LIF scan + crossing ----------
            V = pool.tile([N, T], F32, tag="V")
            nc.vector.tensor_tensor_scan(
                out=V[:, :], data0=alph[:, :], data1=s[:, :], initial=0.0,
                op0=OP.mult, op1=OP.add)
            rm = pool.tile([N, T], F32, tag="rm")
            nc.vector.tensor_tensor_scan(
                out=rm[:, :], data0=V[:, :], data1=V[:, :], initial=-3.0e38,
                op0=OP.max, op1=OP.max)
            lat = pool.tile([N, 1], F32, tag="lat")
            nc.vector.tensor_scalar(
                out=rm[:, :], in0=rm[:, :], scalar1=THRESH, scalar2=0.0,
                op0=OP.is_lt, op1=OP.add, accum_out=lat[:, :])
            nc.scalar.dma_start(out=lat_out[:, :], in_=lat[:, :])

            # ---------- act = exp(-lat/scale), gates + MLP ----------
            # whole MLP stays in [(b, feature), 1] column layout: contraction
            # runs over partitions via block-diagonal / half-zero lhsT packs.
            actv = pool.tile([N, 1], F32, tag="actv")
            nc.scalar.activation(out=actv[:, :], in_=lat[:, :], func=AF.Exp,
                                 scale=-1.0 / scale_val)
            nc.scalar.dma_start(out=act_out[:, :], in_=actv[:, :])

            ps_m = psum.tile([N, 1], F32, tag="psA", name="ps_m")
            nc.tensor.matmul(ps_m[:, :], wm[:, 0:128], actv[:, :],
                             start=True, stop=True)
            mixv = pool.tile([N, 1], F32, tag="mixv")
            nc.scalar.copy(out=mixv[:, :], in_=ps_m[:, :])
            hs = []
            for b in range(NB):
                ps_h = psum.tile([DEC, 1], F32, tag="psB", name=f"ps_h{b}")
                nc.tensor.matmul(ps_h[:, :], wm[:, 128 + b * 128:256 + b * 128],
                                 mixv[:, :], start=True, stop=True)
                hv = pool.tile([DEC, 1], F32, tag=f"hv{b}", name=f"hv{b}")
                nc.scalar.activation(out=hv[:, :], in_=ps_h[:, :],
                                     func=AF.Relu,
                                     bias=pp[:, PP_MB1:PP_MB1 + 1], scale=1.0)
                hs.append(hv)
            ps_r = psum.tile([N, 1], F32, tag="psA", name="ps_r")
            for b in range(NB):
                nc.tensor.matmul(ps_r[b * C:(b + 1) * C, :], wm[:, 384:448],
                                 hs[b][:, :], start=True, stop=True)
            # softplus(raw) = relu(raw) + P(exp(-|raw|)), raw = psum + b2,
            # with P a degree-6 minimax fit of ln(1+u) on [0,1] (err ~1.5e-6)
            # evaluated on VectorE so no Ln table is ever loaded
            PC = [1.4698117483186821e-06, 0.9998477529839347,
                  -0.4973735992304131, 0.31574842159222594,
                  -0.1903558305283913, 0.0826921540715837,
                  -0.01741427410397167]
            absT = pool.tile([N, 1], F32, tag="absT")
            nc.scalar.activation(out=absT[:, :], in_=ps_r[:, :], func=AF.Abs,
                                 bias=pp[:, PP_MB2:PP_MB2 + 1], scale=1.0)
            expT = pool.tile([N, 1], F32, tag="expT")
            nc.scalar.activation(out=expT[:, :], in_=absT[:, :], func=AF.Exp,
                                 scale=-1.0)
            reluT = pool.tile([N, 1], F32, tag="reluT")
            nc.vector.tensor_scalar(
                out=reluT[:, :], in0=ps_r[:, :],
                scalar1=pp[:, PP_MB2:PP_MB2 + 1], scalar2=0.0,
                op0=OP.add, op1=OP.max)
            ply = pool.tile([N, 1], F32, tag="ply")
            nc.vector.tensor_scalar(out=ply[:, :], in0=expT[:, :],
                                    scalar1=PC[6], scalar2=PC[5],
                                    op0=OP.mult, op1=OP.add)
            for k in (4, 3, 2, 1, 0):
                nc.vector.tensor_mul(out=ply[:, :], in0=ply[:, :],
                                     in1=expT[:, :])
                nc.vector.tensor_scalar_add(out=ply[:, :], in0=ply[:, :],
                                            scalar1=PC[k])
            predT = pool.tile([N, 1], F32, tag="predT")
            nc.vector.tensor_add(out=predT[:, :], in0=reluT[:, :],
                                 in1=ply[:, :])
            nc.vector.tensor_scalar(
                out=predT[:, :], in0=predT[:, :], scalar1=0.0, scalar2=float(T),
                op0=OP.max, op1=OP.min)
            nc.scalar.dma_start(out=pred_out[:, :], in_=predT[:, :])

    nc.compile()
    return nc


def _host_prep(w3, b3, w5, b5, reduce_w, reduce_b, output_gates,
               mlp_w1, mlp_b1, mlp_w2, mlp_b2):
    cidx = np.arange(C)
    one_m_a = np.float32(1.0 - ALPHA)

    def per_row(vec_c):
        return np.tile(np.asarray(vec_c, np.float32), NB)

    # effective 5-tap summed kernel: conv paths mixed with reduce_w, then
    # scaled by (1-alpha) so the scan's data1 is the scaled drive directly
    eff_w = np.zeros((C, 5), np.float32)
    # conv3 taps sit at positions 1..3 of the 5-tap window
    for j in range(K1):
        eff_w[:, j + 1] += reduce_w[:, 0] * w3[2 * cidx + 0, 0, j]
        eff_w[:, j + 1] += reduce_w[:, 1] * w3[2 * cidx + 1, 0, j]
    for j in range(K2):
        eff_w[:, j] += reduce_w[:, 2] * w5[2 * cidx + 0, 0, j]
        eff_w[:, j] += reduce_w[:, 3] * w5[2 * cidx + 1, 0, j]
    eff_b = (reduce_w[:, 0] * b3[2 * cidx + 0] +
             reduce_w[:, 1] * b3[2 * cidx + 1] +
             reduce_w[:, 2] * b5[2 * cidx + 0] +
             reduce_w[:, 3] * b5[2 * cidx + 1] + reduce_b)
    eff_w *= one_m_a
    eff_b = eff_b * one_m_a

    pp = np.zeros((N, PP_NCOL), np.float32)
    for j in range(5):
        pp[:, PP_EW + j] = per_row(eff_w[:, j])
    pp[:, PP_EB] = per_row(eff_b)
    pp[:, PP_MB1] = np.asarray(mlp_b1, np.float32)
    pp[:, PP_MB2] = per_row(mlp_b2)
    pp[0:C, PP_BA] = b3[2 * cidx + 0]
    pp[C:N, PP_BA] = b3[2 * cidx + 1]
    pp[0:C, PP_BB] = b5[2 * cidx + 0]
    pp[C:N, PP_BB] = b5[2 * cidx + 1]

    # MLP lhsT packs for the [(b, feature), 1] column formulation:
    #   cols 0:128    block-diag over b of output_gates.T  (act -> mixed)
    #   cols 128:256  w1.T on rows (b=0, ci), zeros elsewhere
    #   cols 256:384  w1.T on rows (b=1, ci)
    #   cols 384:448  w2.T (contraction over all DEC rows)
    wm = np.zeros((N, 448), np.float32)
    gT = np.ascontiguousarray(output_gates.T)
    wm[0:C, 0:C] = gT
    wm[C:N, C:2 * C] = gT
    wm[0:C, 128:256] = np.ascontiguousarray(mlp_w1.T)
    wm[C:N, 256:384] = np.ascontiguousarray(mlp_w1.T)
    wm[:, 384:448] = np.ascontiguousarray(mlp_w2.T)

    # conv matmul lhsT stack [128, 10*128]: out = lhsT.T @ rhs.
    # Blocks 0..4 (b=0): rhs row k<64 is x[b, k, t+s], row 64+k the
    # one-shifted copy. Blocks 5..9 (b=1) are the row-half-swapped
    # versions (plain rows live on partitions 64..127 for b=1).
    # out col m<64: path A0/B0 channel m; col 64+m: path A1/B1 channel m.
    wc = np.zeros((N, 10 * 128), np.float32)

    def put(mm, krow, mcol, val):
        wc[krow, mm * 128 + mcol] = val

    for c in range(C):
        # mm0: A-set, s=-1: top tap j=0, bottom tap j=1
        put(0, c, c, w3[2 * c + 0, 0, 0])
        put(0, c, 64 + c, w3[2 * c + 1, 0, 0])
        put(0, 64 + c, c, w3[2 * c + 0, 0, 1])
        put(0, 64 + c, 64 + c, w3[2 * c + 1, 0, 1])
        # mm1: A-set, s=+1: top tap j=2
        put(1, c, c, w3[2 * c + 0, 0, 2])
        put(1, c, 64 + c, w3[2 * c + 1, 0, 2])
        # mm2: B-set, s=-2: top j=0, bottom j=1
        put(2, c, c, w5[2 * c + 0, 0, 0])
        put(2, c, 64 + c, w5[2 * c + 1, 0, 0])
        put(2, 64 + c, c, w5[2 * c + 0, 0, 1])
        put(2, 64 + c, 64 + c, w5[2 * c + 1, 0, 1])
        # mm3: B-set, s=0: top j=2, bottom j=3
        put(3, c, c, w5[2 * c + 0, 0, 2])
        put(3, c, 64 + c, w5[2 * c + 1, 0, 2])
        put(3, 64 + c, c, w5[2 * c + 0, 0, 3])
        put(3, 64 + c, 64 + c, w5[2 * c + 1, 0, 3])
        # mm4: B-set, s=+2: top j=4
        put(4, c, c, w5[2 * c + 0, 0, 4])
        put(4, c, 64 + c, w5[2 * c + 1, 0, 4])
    # b=1 blocks: swap the row halves of each lhsT
    for mm in range(5):
        blk = wc[:, mm * 128:(mm + 1) * 128]
        wc[0:C, (5 + mm) * 128:(6 + mm) * 128] = blk[C:N, :]
        wc[C:N, (5 + mm) * 128:(6 + mm) * 128] = blk[0:C, :]

    return pp, wm, wc


def _run(inputs, trace=False):
    x = np.asarray(inputs["x"], np.float32)
    scale_val = max(float(np.asarray(inputs["latency_scale"])), 0.001)
    pp, wm, wc = _host_prep(
        np.asarray(inputs["w3"], np.float32), np.asarray(inputs["b3"], np.float32),
        np.asarray(inputs["w5"], np.float32), np.asarray(inputs["b5"], np.float32),
        np.asarray(inputs["reduce_w"], np.float32),
        np.asarray(inputs["reduce_b"], np.float32),
        np.asarray(inputs["output_gates"], np.float32),
        np.asarray(inputs["mlp_w1"], np.float32),
        np.asarray(inputs["mlp_b1"], np.float32),
        np.asarray(inputs["mlp_w2"], np.float32),
        np.asarray(inputs["mlp_b2"], np.float32))

    key = round(scale_val, 9)
    if key not in _GRAPH_CACHE:
        _GRAPH_CACHE[key] = _build_graph(scale_val)
    nc = _GRAPH_CACHE[key]

    in_maps = []
    for k in range(N_CORES):
        xs = np.ascontiguousarray(
            x[k * NB:(k + 1) * NB].reshape(N, T), dtype=np.float32)
        in_maps.append({"x": xs, "pp": pp, "wm": wm, "wc": wc})

    res = run_bass_kernel_spmd(nc, in_maps, list(range(N_CORES)), trace=trace)

    pred = np.concatenate(
        [res.results[k]["pred"].reshape(NB, C) for k in range(N_CORES)], axis=0)
    lat = np.concatenate(
        [res.results[k]["lat"].reshape(NB, C) for k in range(N_CORES)], axis=0)
    act = np.concatenate(
        [res.results[k]["act"].reshape(NB, C) for k in range(N_CORES)], axis=0)
    dd = np.concatenate(
        [res.results[k]["dd"].reshape(NB, C, D, T) for k in range(N_CORES)],
        axis=0)
    outs = (pred.astype(np.float32), lat.astype(np.float32),
            act.astype(np.float32), dd.astype(np.float32))
    return outs, res


def kernel(**inputs):
    outs, _ = _run(inputs, trace=False)
    return outs


# revision 46
# speedup vs baseline: 1.2852x; 1.1319x over previous
"""Trainium2 Bass kernel for nn_ChannelLatencySeq2Value.

Data-parallel over B across 8 NeuronCores (2 batch rows / core).

Per core:
  - dendritic_drive (4 depthwise conv paths) runs on the TensorEngine as
    block-diagonal bf16 matmuls in [c, (b, t)] layout. Tap pairs are packed
    onto the 128-row contraction dim by storing a one-column-shifted copy of
    x on partitions 64..127, so the 16 taps collapse into 5 matmul passes.
    ScalarE copies PSUM->SBUF adding the conv bias; results DMA straight out.
  - summed (the LIF drive) stays exact fp32 on VectorE in [(b,c), t] layout:
    a 5-tap effective kernel (conv paths pre-mixed with reduce_w host-side),
    then one tensor_tensor_scan for the LIF recurrence, a running-max scan,
    and a fused compare+accumulate for the first-crossing latency.
  - act/gates/MLP run as [(b, feature), 1] column matmuls (block-diagonal
    lhsT packs), softplus via exp + a degree-6 ln(1+u) polynomial on VectorE
    so only one ACT table set is ever loaded.

All parameters are tiny and replicated; no collectives are needed.
Measured: ~88-91 us HW exec (8 cores), outputs <= 2e-3 rel err (bf16 conv),
true_latency bit-exact vs an fp32 reference scan.
"""

import numpy as np

import concourse.bass as bass
import concourse.bacc as bacc
import concourse.mybir as mybir
from concourse import tile
from concourse.bass_utils import run_bass_kernel_spmd

F32 = mybir.dt.float32
BF16 = mybir.dt.bfloat16
OP = mybir.AluOpType
AF = mybir.ActivationFunctionType

B, C, T = 16, 64, 4096
K1, O1 = 3, 2
K2, O2 = 5, 2
D = O1 + O2
DEC = 128
TAU, THRESH = 5.0, 1.0
ALPHA = float(np.exp(-1.0 / TAU))
N_CORES = 8
NB = B // N_CORES          # batch rows per core
N = NB * C                 # partition rows per core (= 128)
TP = T + 4                 # per-b padded width (2 zeros each side)
FCH = 512                  # matmul moving chunk (one PSUM bank)
HCH = 1024                 # staging chunk (2 banks; psA+psB double-buffered)

# pp column layout (per-partition parameter pack)
PP_EW = 0                  # effective summed kernel taps, cols 0..4
PP_EB = 5                  # effective summed bias
PP_MB1 = 6                 # mlp_b1 (row = DEC index)
PP_MB2 = 7                 # mlp_b2 (rows 0..C-1)
PP_BA = 8                  # conv bias for A-set rows (dd0|dd1)
PP_BB = 9                  # conv bias for B-set rows (dd2|dd3)
PP_NCOL = 10

# matmul tap shift bases (rhs col offset = 2 + t0 + s)
MM_SHIFT = [-1, 1, -2, 0, 2]   # mm0,mm1 -> A-set; mm2..mm4 -> B-set
MM_SET = [0, 0, 1, 1, 1]

_GRAPH_CACHE = {}


def _build_graph(scale_val: float):
    nc = bacc.Bacc("TRN2", target_bir_lowering=False, debug=False,
                   num_devices=N_CORES)

    x_in = nc.declare_dram_parameter("x", [N, T], F32, isOutput=False)
    pp_in = nc.declare_dram_parameter("pp", [N, PP_NCOL], F32, isOutput=False)
    wm_in = nc.declare_dram_parameter("wm", [N, 448], F32, isOutput=False)
    wc_in = nc.declare_dram_parameter("wc", [N, 10 * 128], F32, isOutput=False)
    dd_out = nc.declare_dram_parameter("dd", [N, D, T], F32, isOutput=True)
    lat_out = nc.declare_dram_parameter("lat", [N, 1], F32, isOutput=True)
    act_out = nc.declare_dram_parameter("act", [N, 1], F32, isOutput=True)
    pred_out = nc.declare_dram_parameter("pred", [N, 1], F32, isOutput=True)

    HLF = T // 2
    with tile.TileContext(nc) as tc:
        with (
            tc.tile_pool(name="pool", bufs=1) as pool,
            tc.tile_pool(name="stage", bufs=4) as stage,
            tc.tile_pool(name="psum", bufs=2, space="PSUM") as psum,
        ):
            # ---------- input DMAs (one queue, priority order) ----------
            pp = pool.tile([N, PP_NCOL], F32, tag="pp")
            wm = pool.tile([N, 448], F32, tag="wm")
            wcf = pool.tile([N, 10 * 128], F32, tag="wcf")
            nc.sync.dma_start(out=pp[:, :], in_=pp_in[:, :])
            nc.sync.dma_start(out=wcf[:, :], in_=wc_in[:, :])

            xsA = pool.tile([N, HLF + 4], F32, tag="xsA")
            xsB = pool.tile([N, HLF + 4], F32, tag="xsB")
            nc.vector.memset(xsA[:, 0:2], 0.0)
            nc.sync.dma_start(out=xsA[:, 2:HLF + 4], in_=x_in[:, 0:HLF + 2])
            nc.scalar.dma_start(out=xsB[:, 0:HLF + 2], in_=x_in[:, HLF - 2:T])
            nc.vector.memset(xsB[:, HLF + 2:HLF + 4], 0.0)
            nc.sync.dma_start(out=wm[:, :], in_=wm_in[:, :])

            # warm the exp ACT table set (covers identity/abs/relu/copy too);
            # no other set is ever needed, so no tail table switches
            warm = pool.tile([N, 1], F32, tag="warm")
            nc.vector.memset(warm[:, :], 0.0)
            nc.scalar.activation(out=warm[:, :], in_=warm[:, :], func=AF.Exp)
            wcb = pool.tile([N, 10 * 128], BF16, tag="wcb")
            nc.scalar.copy(out=wcb[:, :], in_=wcf[:, :])

            # ---------- x2b built straight from the xs halves -------------
            # x2b0: rows 0..63 = b0 channels (cast), rows 64..127 = the same
            # shifted left one column (SB2SB). x2b1 mirrored: plain rows live
            # on 64..127 (partition-aligned with xs), shifted copy on 0..63;
            # its matmuls use the row-swapped weight stack.
            s = pool.tile([N, T], F32, tag="s")
            x2bs = []
            XH = HLF + 4  # xsA/xsB width; cast1 covers cols [0, XH)
            for b in range(NB):
                x2b = pool.tile([N, TP], BF16, tag=f"x2b{b}", name=f"x2b{b}")
                src = slice(0, C) if b == 0 else slice(C, N)
                pl = slice(0, C) if b == 0 else slice(C, N)   # plain rows
                sh = slice(C, N) if b == 0 else slice(0, C)   # shifted rows
                nc.scalar.copy(out=x2b[pl, 0:XH], in_=xsA[src, 0:XH])
                nc.scalar.copy(out=x2b[pl, XH:TP], in_=xsB[src, 4:HLF + 4])
                nc.vector.memset(x2b[sh, TP - 1:TP], 0.0)
                nc.scalar.dma_start(out=x2b[sh, 0:XH - 1],
                                      in_=x2b[pl, 1:XH])
                nc.scalar.dma_start(out=x2b[sh, XH - 1:TP - 1],
                                      in_=x2b[pl, XH:TP])
                x2bs.append(x2b)

            alph = pool.tile([N, T], F32, tag="alph")
            nc.gpsimd.memset(alph[:, :], ALPHA)

            # ---------- summed taps (fp32, VectorE; tap0 via 2x TS) ----------
            for xsh, o in ((xsA, 0), (xsB, HLF)):
                nc.vector.tensor_scalar(
                    out=s[:, o:o + HLF], in0=xsh[:, 0:HLF],
                    scalar1=pp[:, PP_EW:PP_EW + 1],
                    scalar2=pp[:, PP_EB:PP_EB + 1],
                    op0=OP.mult, op1=OP.add)
                for j in range(1, 5):
                    nc.vector.scalar_tensor_tensor(
                        out=s[:, o:o + HLF], in0=xsh[:, j:j + HLF],
                        scalar=pp[:, PP_EW + j:PP_EW + j + 1],
                        in1=s[:, o:o + HLF], op0=OP.mult, op1=OP.add)

            # ---------- dendritic drive via TensorE ----------
            # groups of (b, T-half); stores stream out per group so the
            # 8MiB writeback overlaps the rest of the matmul work
            THL = T // 2
            dd_n = 0
            for b, th in ((0, 0), (1, 0), (0, 1), (1, 1)):
                sbs = [stage.tile([N, THL], F32, tag=f"stg{si}",
                                  name=f"ddsb{b}{th}{si}") for si in range(2)]
                for hh in range(THL // HCH):
                    t0 = th * THL + hh * HCH
                    psA = psum.tile([N, HCH], F32, tag="psA", name="psA")
                    psB = psum.tile([N, HCH], F32, tag="psB", name="psB")
                    for q in range(HCH // FCH):
                        for mm in range(5):
                            ps = psA if MM_SET[mm] == 0 else psB
                            first = mm == 0 or mm == 2
                            last = mm == 1 or mm == 4
                            s_ = MM_SHIFT[mm]
                            c0 = 2 + t0 + q * FCH + s_
                            mmi = mm + 5 * b
                            nc.tensor.matmul(
                                ps[:, q * FCH:(q + 1) * FCH],
                                wcb[:, mmi * 128:(mmi + 1) * 128],
                                x2bs[b][:, c0:c0 + FCH],
                                start=first, stop=last)
                    for si, ps, bcol in ((0, psA, PP_BA), (1, psB, PP_BB)):
                        nc.scalar.activation(
                            out=sbs[si][:, hh * HCH:(hh + 1) * HCH],
                            in_=ps[:, :], func=AF.Identity,
                            bias=pp[:, bcol:bcol + 1], scale=1.0)
                for si in range(2):
                    for phalf in range(2):
                        eng = nc.sync if dd_n % 2 == 0 else nc.scalar
                        dd_n += 1
                        eng.dma_start(
                            out=dd_out[b * C:(b + 1) * C, 2 * si + phalf,
                                       th * THL:(th + 1) * THL],
                            in_=sbs[si][phalf * C:(phalf + 1) * C, :])

            # ---------- LIF scan + crossing ----------
            V = pool.tile([N, T], F32, tag="V")
            nc.vector.tensor_tensor_scan(
                out=V[:, :], data0=alph[:, :], data1=s[:, :], initial=0.0,
                op0=OP.mult, op1=OP.add)
            rm = pool.tile([N, T], F32, tag="rm")
            nc.vector.tensor_tensor_scan(
                out=rm[:, :], data0=V[:, :], data1=V[:, :], initial=-3.0e38,
                op0=OP.max, op1=OP.max)
            lat = pool.tile([N, 1], F32, tag="lat")
            nc.vector.tensor_scalar(
                out=rm[:, :], in0=rm[:, :], scalar1=THRESH, scalar2=0.0,
                op0=OP.is_lt, op1=OP.add, accum_out=lat[:, :])
            nc.scalar.dma_start(out=lat_out[:, :], in_=lat[:, :])

            # ---------- act = exp(-lat/scale), gates + MLP ----------
            # whole MLP stays in [(b, feature), 1] column layout: contraction
            # runs over partitions via block-diagonal / half-zero lhsT packs.
            actv = pool.tile([N, 1], F32, tag="actv")
            nc.scalar.activation(out=actv[:, :], in_=lat[:, :], func=AF.Exp,
                                 scale=-1.0 / scale_val)
            nc.scalar.dma_start(out=act_out[:, :], in_=actv[:, :])

            ps_m = psum.tile([N, 1], F32, tag="psA", name="ps_m")
            nc.tensor.matmul(ps_m[:, :], wm[:, 0:128], actv[:, :],
                             start=True, stop=True)
            mixv = pool.tile([N, 1], F32, tag="mixv")
            nc.scalar.copy(out=mixv[:, :], in_=ps_m[:, :])
            hs = []
            for b in range(NB):
                ps_h = psum.tile([DEC, 1], F32, tag="psB", name=f"ps_h{b}")
                nc.tensor.matmul(ps_h[:, :], wm[:, 128 + b * 128:256 + b * 128],
                                 mixv[:, :], start=True, stop=True)
                hv = pool.tile([DEC, 1], F32, tag=f"hv{b}", name=f"hv{b}")
                nc.scalar.activation(out=hv[:, :], in_=ps_h[:, :],
                                     func=AF.Relu,
                                     bias=pp[:, PP_MB1:PP_MB1 + 1], scale=1.0)
                hs.append(hv)
            ps_r = psum.tile([N, 1], F32, tag="psA", name="ps_r")
            for b in range(NB):
                nc.tensor.matmul(ps_r[b * C:(b + 1) * C, :], wm[:, 384:448],
                                 hs[b][:, :], start=True, stop=True)
            # softplus(raw) = relu(raw) + P(exp(-|raw|)), raw = psum + b2,
            # with P a degree-6 minimax fit of ln(1+u) on [0,1] (err ~1.5e-6)
            # evaluated on VectorE so no Ln table is ever loaded
            PC = [1.4698117483186821e-06, 0.9998477529839347,
                  -0.4973735992304131, 0.31574842159222594,
                  -0.1903558305283913, 0.0826921540715837,
                  -0.01741427410397167]
            absT = pool.tile([N, 1], F32, tag="absT")
            nc.scalar.activation(out=absT[:, :], in_=ps_r[:, :], func=AF.Abs,
                                 bias=pp[:, PP_MB2:PP_MB2 + 1], scale=1.0)
            expT = pool.tile([N, 1], F32, tag="expT")
            nc.scalar.activation(out=expT[:, :], in_=absT[:, :], func=AF.Exp,
                                 scale=-1.0)
            reluT = pool.tile([N, 1], F32, tag="reluT")
            nc.vector.tensor_scalar(
                out=reluT[:, :], in0=ps_r[:, :],
                scalar1=pp[:, PP_MB2:PP_MB2 + 1], scalar2=0.0,
                op0=OP.add, op1=OP.max)
            ply = pool.tile([N, 1], F32, tag="ply")
            nc.vector.tensor_scalar(out=ply[:, :], in0=expT[:, :],
                                    scalar1=PC[6], scalar2=PC[5],
                                    op0=OP.mult, op1=OP.add)
            for k in (4, 3, 2, 1, 0):
                nc.vector.tensor_mul(out=ply[:, :], in0=ply[:, :],
                                     in1=expT[:, :])
                nc.vector.tensor_scalar_add(out=ply[:, :], in0=ply[:, :],
                                            scalar1=PC[k])
            predT = pool.tile([N, 1], F32, tag="predT")
            nc.vector.tensor_add(out=predT[:, :], in0=reluT[:, :],
                                 in1=ply[:, :])
            nc.vector.tensor_scalar(
                out=predT[:, :], in0=predT[:, :], scalar1=0.0, scalar2=float(T),
                op0=OP.max, op1=OP.min)
            nc.scalar.dma_start(out=pred_out[:, :], in_=predT[:, :])

    nc.compile()
    return nc


def _host_prep(w3, b3, w5, b5, reduce_w, reduce_b, output_gates,
               mlp_w1, mlp_b1, mlp_w2, mlp_b2):
    cidx = np.arange(C)
    one_m_a = np.float32(1.0 - ALPHA)

    def per_row(vec_c):
        return np.tile(np.asarray(vec_c, np.float32), NB)

    # effective 5-tap summed kernel: conv paths mixed with reduce_w, then
    # scaled by (1-alpha) so the scan's data1 is the scaled drive directly
    eff_w = np.zeros((C, 5), np.float32)
    # conv3 taps sit at positions 1..3 of the 5-tap window
    for j in range(K1):
        eff_w[:, j + 1] += reduce_w[:, 0] * w3[2 * cidx + 0, 0, j]
        eff_w[:, j + 1] += reduce_w[:, 1] * w3[2 * cidx + 1, 0, j]
    for j in range(K2):
        eff_w[:, j] += reduce_w[:, 2] * w5[2 * cidx + 0, 0, j]
        eff_w[:, j] += reduce_w[:, 3] * w5[2 * cidx + 1, 0, j]
    eff_b = (reduce_w[:, 0] * b3[2 * cidx + 0] +
             reduce_w[:, 1] * b3[2 * cidx + 1] +
             reduce_w[:, 2] * b5[2 * cidx + 0] +
             reduce_w[:, 3] * b5[2 * cidx + 1] + reduce_b)
    eff_w *= one_m_a
    eff_b = eff_b * one_m_a

    pp = np.zeros((N, PP_NCOL), np.float32)
    for j in range(5):
        pp[:, PP_EW + j] = per_row(eff_w[:, j])
    pp[:, PP_EB] = per_row(eff_b)
    pp[:, PP_MB1] = np.asarray(mlp_b1, np.float32)
    pp[:, PP_MB2] = per_row(mlp_b2)
    pp[0:C, PP_BA] = b3[2 * cidx + 0]
    pp[C:N, PP_BA] = b3[2 * cidx + 1]
    pp[0:C, PP_BB] = b5[2 * cidx + 0]
    pp[C:N, PP_BB] = b5[2 * cidx + 1]

    # MLP lhsT packs for the [(b, feature), 1] column formulation:
    #   cols 0:128    block-diag over b of output_gates.T  (act -> mixed)
    #   cols 128:256  w1.T on rows (b=0, ci), zeros elsewhere
    #   cols 256:384  w1.T on rows (b=1, ci)
    #   cols 384:448  w2.T (contraction over all DEC rows)
    wm = np.zeros((N, 448), np.float32)
    gT = np.ascontiguousarray(output_gates.T)
    wm[0:C, 0:C] = gT
    wm[C:N, C:2 * C] = gT
    wm[0:C, 128:256] = np.ascontiguousarray(mlp_w1.T)
    wm[C:N, 256:384] = np.ascontiguousarray(mlp_w1.T)
    wm[:, 384:448] = np.ascontiguousarray(mlp_w2.T)

    # conv matmul lhsT stack [128, 10*128]: out = lhsT.T @ rhs.
    # Blocks 0..4 (b=0): rhs row k<64 is x[b, k, t+s], row 64+k the
    # one-shifted copy. Blocks 5..9 (b=1) are the row-half-swapped
    # versions (plain rows live on partitions 64..127 for b=1).
    # out col m<64: path A0/B0 channel m; col 64+m: path A1/B1 channel m.
    wc = np.zeros((N, 10 * 128), np.float32)

    def put(mm, krow, mcol, val):
        wc[krow, mm * 128 + mcol] = val

    for c in range(C):
        # mm0: A-set, s=-1: top tap j=0, bottom tap j=1
        put(0, c, c, w3[2 * c + 0, 0, 0])
        put(0, c, 64 + c, w3[2 * c + 1, 0, 0])
        put(0, 64 + c, c, w3[2 * c + 0, 0, 1])
        put(0, 64 + c, 64 + c, w3[2 * c + 1, 0, 1])
        # mm1: A-set, s=+1: top tap j=2
        put(1, c, c, w3[2 * c + 0, 0, 2])
        put(1, c, 64 + c, w3[2 * c + 1, 0, 2])
        # mm2: B-set, s=-2: top j=0, bottom j=1
        put(2, c, c, w5[2 * c + 0, 0, 0])
        put(2, c, 64 + c, w5[2 * c + 1, 0, 0])
        put(2, 64 + c, c, w5[2 * c + 0, 0, 1])
        put(2, 64 + c, 64 + c, w5[2 * c + 1, 0, 1])
        # mm3: B-set, s=0: top j=2, bottom j=3
        put(3, c, c, w5[2 * c + 0, 0, 2])
        put(3, c, 64 + c, w5[2 * c + 1, 0, 2])
        put(3, 64 + c, c, w5[2 * c + 0, 0, 3])
        put(3, 64 + c, 64 + c, w5[2 * c + 1, 0, 3])
        # mm4: B-set, s=+2: top j=4
        put(4, c, c, w5[2 * c + 0, 0, 4])
        put(4, c, 64 + c, w5[2 * c + 1, 0, 4])
    # b=1 blocks: swap the row halves of each lhsT
    for mm in range(5):
        blk = wc[:, mm * 128:(mm + 1) * 128]
        wc[0:C, (5 + mm) * 128:(6 + mm) * 128] = blk[C:N, :]
        wc[C:N, (5 + mm) * 128:(6 + mm) * 128] = blk[0:C, :]

    return pp, wm, wc


def _run(inputs, trace=False):
    x = np.asarray(inputs["x"], np.float32)
    scale_val = max(float(np.asarray(inputs["latency_scale"])), 0.001)
    pp, wm, wc = _host_prep(
        np.asarray(inputs["w3"], np.float32), np.asarray(inputs["b3"], np.float32),
        np.asarray(inputs["w5"], np.float32), np.asarray(inputs["b5"], np.float32),
        np.asarray(inputs["reduce_w"], np.float32),
        np.asarray(inputs["reduce_b"], np.float32),
        np.asarray(inputs["output_gates"], np.float32),
        np.asarray(inputs["mlp_w1"], np.float32),
        np.asarray(inputs["mlp_b1"], np.float32),
        np.asarray(inputs["mlp_w2"], np.float32),
        np.asarray(inputs["mlp_b2"], np.float32))

    key = round(scale_val, 9)
    if key not in _GRAPH_CACHE:
        _GRAPH_CACHE[key] = _build_graph(scale_val)
    nc = _GRAPH_CACHE[key]

    in_maps = []
    for k in range(N_CORES):
        xs = np.ascontiguousarray(
            x[k * NB:(k + 1) * NB].reshape(N, T), dtype=np.float32)
        in_maps.append({"x": xs, "pp": pp, "wm": wm, "wc": wc})

    res = run_bass_kernel_spmd(nc, in_maps, list(range(N_CORES)), trace=trace)

    pred = np.concatenate(
        [res.results[k]["pred"].reshape(NB, C) for k in range(N_CORES)], axis=0)
    lat = np.concatenate(
        [res.results[k]["lat"].reshape(NB, C) for k in range(N_CORES)], axis=0)
    act = np.concatenate(
        [res.results[k]["act"].reshape(NB, C) for k in range(N_CORES)], axis=0)
    dd = np.concatenate(
        [res.results[k]["dd"].reshape(NB, C, D, T) for k in range(N_CORES)],
        axis=0)
    outs = (pred.astype(np.float32), lat.astype(np.float32),
            act.astype(np.float32), dd.astype(np.float32))
    return outs, res


def kernel(**inputs):
    outs, _ = _run(inputs, trace=False)
    return outs
